# revision 10
# baseline (speedup 1.0000x reference)
"""Trainium2 Bass kernel for nn_EnhancedCryptoTransformer (8-layer post-LN
transformer, B=32 S=250 D=512 H=16 DFF=2048, gated attention blend, causal
exponential time-decay additive bias).

Sharding: pure data-parallel over batch - 4 sequences per NeuronCore, no
collectives.  Per-core activations are kept feature-major [D, T] (T=1000
tokens) so every GEMM uses natural-layout weights as the PE stationary
operand.  Attention computes scores^T per (batch, head) so softmax
normalizers fall out of the AV matmul via a ones-column interleaved into the
token-major V tiles.  LayerNorm statistics use PE ones-matmul partition
reductions; per-token stats are broadcast back across partitions with
indicator-matrix matmuls.
"""

import functools
import math
import os
import sys

sys.path.insert(0, "/opt/trn_rl_repo")

import numpy as np
import ml_dtypes

import concourse.bacc as bacc
import concourse.bass as bass
import concourse.mybir as mybir
import concourse.tile as tile
from concourse.bass_utils import run_bass_kernel_spmd

F32 = mybir.dt.float32
F32R = mybir.dt.float32r
BF16 = mybir.dt.bfloat16
AF = mybir.ActivationFunctionType
ALU = mybir.AluOpType
BF16NP = ml_dtypes.bfloat16

B, S, D, H, L, DFF, IN = 32, 250, 512, 16, 8, 2048, 64
DK = D // H                      # 32
NCORES = 8
BC = B // NCORES                 # 4 sequences per core
T = BC * S                       # 1000 tokens per core
TP = 1008                        # padded free size
TCH = TP // 2                    # 504 t-chunk for GEMMs / LN
FCH = TP // 4                    # 252 t-chunk for FFN
NM = D // 128                    # 4 partition tiles per [D, T] tensor
NV = T // 125                    # 8 token-major v tiles of 125 tokens
EPS = 1e-5

N_LAYERS = int(os.environ.get("KERNEL_LAYERS", L))

# pvec column map (per-partition scalars, packed [4, 128, 16] per layer)
PC_BQ, PC_BK, PC_BO, PC_FB2 = 0, 1, 2, 3
PC_LAS, PC_LANS, PC_LAB = 4, 5, 6
PC_N1S, PC_N1NS, PC_N1B = 7, 8, 9
PC_N2S, PC_N2NS, PC_N2B = 10, 11, 12
PC_GB1, PC_GB2 = 13, 14
NPC = 16


def _build_program(n_layers: int):
    nc = bacc.Bacc("TRN2", target_bir_lowering=False, debug=False)

    # ---------------- DRAM I/O ----------------
    xT_d = nc.dram_tensor("xT", [IN, TP], F32, kind="ExternalInput")
    win_d = nc.dram_tensor("win", [IN, D], F32, kind="ExternalInput")
    add_d = nc.dram_tensor("addpos", [NM, 128, S], F32, kind="ExternalInput")
    ps_d = nc.dram_tensor("psrep", [128, S], F32, kind="ExternalInput")
    etb_d = nc.dram_tensor("etb", [2, 128, 256], BF16, kind="ExternalInput")
    e128_d = nc.dram_tensor("e128", [128, 128], BF16, kind="ExternalInput")
    fpvec_d = nc.dram_tensor("fpvec", [NM, 128, 4], F32, kind="ExternalInput")
    out_d = nc.dram_tensor("out", [NM, 128, T], F32, kind="ExternalOutput")

    wl = []
    for l in range(n_layers):
        wl.append(dict(
            wq=nc.dram_tensor(f"L{l}_wq", [D, D], BF16, kind="ExternalInput"),
            wk=nc.dram_tensor(f"L{l}_wk", [D, D], BF16, kind="ExternalInput"),
            wv=nc.dram_tensor(f"L{l}_wv", [D, D], BF16, kind="ExternalInput"),
            wo=nc.dram_tensor(f"L{l}_wo", [D, D], BF16, kind="ExternalInput"),
            g1=nc.dram_tensor(f"L{l}_g1", [D, D // 2], BF16, kind="ExternalInput"),
            g2=nc.dram_tensor(f"L{l}_g2", [D // 2, 1], BF16, kind="ExternalInput"),
            f1=nc.dram_tensor(f"L{l}_f1", [D, DFF], BF16, kind="ExternalInput"),
            f2=nc.dram_tensor(f"L{l}_f2", [DFF, D], BF16, kind="ExternalInput"),
            pvec=nc.dram_tensor(f"L{l}_pvec", [NM, 128, NPC], F32, kind="ExternalInput"),
            fb1=nc.dram_tensor(f"L{l}_fb1", [128, 16], F32, kind="ExternalInput"),
            bvrep=nc.dram_tensor(f"L{l}_bvrep", [128, D], BF16, kind="ExternalInput"),
        ))

    with tile.TileContext(nc) as tc:
        import contextlib
        ctx = contextlib.ExitStack()
        with ctx:
            # ---------------- pools ----------------
            per = ctx.enter_context(tc.tile_pool(name="per", bufs=1))          # persistent
            wq_p = ctx.enter_context(tc.tile_pool(name="wq_p", bufs=1))        # weights (tags manage slots)
            sq_p = ctx.enter_context(tc.tile_pool(name="sq_p", bufs=1))
            ps_big = ctx.enter_context(tc.tile_pool(name="ps_big", bufs=1, space="PSUM"))
            ps_scav = ctx.enter_context(tc.tile_pool(name="ps_scav", bufs=1, space="PSUM"))
            ps_small = ctx.enter_context(tc.tile_pool(name="ps_small", bufs=1, space="PSUM"))

            def ptile(shape, dt, name, pool=per, tag=None, bufs=1):
                return pool.tile(shape, dt, name=name, tag=tag or name, bufs=bufs)

            # ---------------- persistent big buffers ----------------
            h_a = [ptile([128, TP], F32, f"h_a{m}") for m in range(NM)]
            h_b = [ptile([128, TP], F32, f"h_b{m}") for m in range(NM)]
            P = [ptile([128, TP], F32, f"P{m}") for m in range(NM)]       # blend / final-LN out
            h1 = [ptile([128, TP], F32, f"h1{m}") for m in range(NM)]
            h_bf = [ptile([128, TP], BF16, f"hbf{m}") for m in range(NM)]
            h1b = h_bf  # reused: h_bf is dead once the gate GEMM has consumed it
            q_t = [ptile([128, TP], BF16, f"q{m}") for m in range(NM)]
            k_t = [ptile([128, TP], BF16, f"k{m}") for m in range(NM)]
            ctx_t = [ptile([128, TP], BF16, f"ctx{m}") for m in range(NM)]
            v_t = [ptile([128, H * 33], BF16, f"v{m}") for m in range(NV)]   # [125 tokens, 16*(32+1)]
            den_t = [ptile([128, TP], BF16, f"den{m}") for m in range(NM)]

            # ---------------- constants ----------------
            ones128 = ptile([128, 1], F32, "ones128")
            nc.vector.memset(ones128[:], 1.0)
            ones1 = ptile([1, 128], F32, "ones1")
            nc.vector.memset(ones1[:], 1.0)
            ones128b = ptile([128, 1], BF16, "ones128b")
            nc.vector.memset(ones128b[:], 1.0)
            eps_t = ptile([1, 1], F32, "eps_t")
            nc.vector.memset(eps_t[:], EPS)
            e128_t = ptile([128, 128], BF16, "e128_t")
            nc.sync.dma_start(e128_t[:], e128_d[:])
            etb_t = [ptile([128, 256], BF16, f"etb{c}") for c in range(2)]
            for c in range(2):
                nc.sync.dma_start(etb_t[c][:], etb_d[c])
            fpv_t = [ptile([128, 4], F32, f"fpv{m}") for m in range(NM)]
            for m in range(NM):
                nc.sync.dma_start(fpv_t[m][:], fpvec_d[m])

            # init: v ones-columns, denominators, pad columns of h_a
            for i in range(NV):
                nc.vector.memset(v_t[i][:125].rearrange("p (h c) -> p h c", h=H)[:, :, 32:33], 1.0)
            for m in range(NM):
                nc.vector.memset(den_t[m][:], 1.0)
                nc.vector.memset(ctx_t[m][:, T:TP], 0.0)
                nc.vector.memset(h_a[m][:, T:TP], 0.0)
                nc.vector.memset(h_b[m][:, T:TP], 0.0)
                nc.vector.memset(h1[m][:, T:TP], 0.0)
                nc.vector.memset(P[m][:, T:TP], 0.0)

            def r32(ap):
                return ap.bitcast(F32R)

            # ---------------- input projection + positional ----------------
            xT_t = [sq_p.tile([64, TCH], F32, name=f"xT{i}", tag="sq", bufs=3) for i in range(2)]
            for i in range(2):
                nc.sync.dma_start(xT_t[i][:], xT_d[:, i * TCH:(i + 1) * TCH])
            win_t = [wq_p.tile([IN, 128], F32, name=f"win{m}", tag="wqkvo", bufs=8) for m in range(NM)]
            for m in range(NM):
                nc.sync.dma_start(win_t[m][:], win_d[:, m * 128:(m + 1) * 128])
            for m in range(NM):
                for tci in range(2):
                    acc = ps_big.tile([128, TCH], F32, name="accin", tag="big", bufs=2)
                    nc.tensor.matmul(acc[:], win_t[m][:], xT_t[tci][:], start=True, stop=True)
                    nc.scalar.copy(h_a[m][:, tci * TCH:(tci + 1) * TCH], acc[:])
            ps_t = sq_p.tile([128, S], F32, name="ps_t", tag="sq", bufs=3)
            nc.sync.dma_start(ps_t[:], ps_d[:])
            for m in range(NM):
                add_t = sq_p.tile([128, S], F32, name=f"add{m}", tag="sq", bufs=3)
                nc.sync.dma_start(add_t[:], add_d[m])
                for b in range(BC):
                    sl = slice(b * S, (b + 1) * S)
                    nc.vector.tensor_mul(h_a[m][:, sl], h_a[m][:, sl], ps_t[:])
                    nc.vector.tensor_add(h_a[m][:, sl], h_a[m][:, sl], add_t[:])

            # ---------------- helpers ----------------
            def load_w(dram, kparts, ncols, dt, tag, bufs, name):
                """load [K, N] dram weight into kparts tiles of [128, ncols]."""
                ts = []
                for kc in range(kparts):
                    wt = wq_p.tile([128, ncols], dt, name=f"{name}_{kc}", tag=tag, bufs=bufs)
                    nc.sync.dma_start(wt[:], dram[kc * 128:(kc + 1) * 128, :])
                    ts.append(wt)
                return ts

            def gemm_fm_bf16(w_tiles, rhs_tiles, nm_out, out_cb, kparts=NM, tch=TCH):
                """feature-major GEMM: out[m*128:(m+1)*128, tchunk] tiles via PSUM.
                w_tiles: kparts tiles [128, nm_out*128] bf16 (lhsT slices taken per m)
                rhs_tiles: kparts activation tiles [128, TP] bf16
                out_cb(m, tci, acc_psum): consume psum [128, tch]"""
                nchunks = TP // tch
                for m in range(nm_out):
                    for tci in range(nchunks):
                        sl = slice(tci * tch, (tci + 1) * tch)
                        acc = ps_big.tile([128, tch], F32, name="acc", tag="big", bufs=2)
                        for kc in range(kparts):
                            nc.tensor.matmul(
                                acc[:], w_tiles[kc][:, m * 128:(m + 1) * 128],
                                rhs_tiles[kc][:, sl],
                                start=(kc == 0), stop=(kc == kparts - 1))
                        out_cb(m, tci, sl, acc)

            def emit_ln(x_tiles, out_tiles, pv_idx, pv_tiles=None):
                """feature-major LayerNorm over D: out = (x-m)/sd*gamma+beta.
                pv_idx = (s_col, negs_col, b_col); pv_tiles: list of [128, NPC] tiles."""
                s_col, ns_col, b_col = pv_idx
                for tci in range(2):
                    sl = slice(tci * TCH, (tci + 1) * TCH)
                    ssum = ps_small.tile([1, TCH], F32, name="ssum", tag="small", bufs=2)
                    ssq = ps_small.tile([1, TCH], F32, name="ssq", tag="small", bufs=2)
                    for m in range(NM):
                        xb = sq_p.tile([128, TCH], BF16, name="xb", tag="sqb", bufs=6)
                        nc.gpsimd.tensor_copy(xb[:], x_tiles[m][:, sl])
                        sqb = sq_p.tile([128, TCH], BF16, name="sqb", tag="sqb", bufs=6)
                        nc.gpsimd.tensor_mul(sqb[:], xb[:], xb[:])
                        nc.tensor.matmul(ssum[:], ones128b[:], xb[:],
                                         start=(m == 0), stop=(m == NM - 1))
                        nc.tensor.matmul(ssq[:], ones128b[:], sqb[:],
                                         start=(m == 0), stop=(m == NM - 1))
                    a0 = sq_p.tile([1, TCH], F32, name="a0", tag="lns", bufs=5)
                    a1 = sq_p.tile([1, TCH], F32, name="a1", tag="lns", bufs=5)
                    v1 = sq_p.tile([1, TCH], F32, name="v1", tag="lns", bufs=5)
                    nc.scalar.activation(a0[:], ssum[:], AF.Copy, scale=1.0 / D)
                    nc.scalar.activation(a1[:], ssq[:], AF.Copy, scale=1.0 / D)
                    nc.vector.tensor_mul(v1[:], a0[:], a0[:])
                    nc.vector.tensor_sub(v1[:], a1[:], v1[:])
                    nc.scalar.activation(v1[:], v1[:], AF.Sqrt, bias=eps_t[:])
                    nc.vector.reciprocal(v1[:], v1[:])           # r = 1/sd
                    nc.vector.tensor_mul(a0[:], a0[:], v1[:])    # mr = mean * r
                    rb = ps_small.tile([128, TCH], F32, name="rb", tag="small", bufs=2)
                    mrb = ps_small.tile([128, TCH], F32, name="mrb", tag="small", bufs=2)
                    nc.tensor.matmul(rb[:], ones1[:], v1[:], start=True, stop=True)
                    nc.tensor.matmul(mrb[:], ones1[:], a0[:], start=True, stop=True)
                    for m in range(NM):
                        pv = pv_tiles[m]
                        d_t = sq_p.tile([128, TCH], F32, name="d_t", tag="d_t", bufs=2)
                        nc.scalar.activation(d_t[:], mrb[:], AF.Identity,
                                             bias=pv[:, b_col:b_col + 1],
                                             scale=pv[:, ns_col:ns_col + 1])
                        nc.vector.scalar_tensor_tensor(
                            out_tiles[m][:, sl], rb[:], pv[:, s_col:s_col + 1],
                            x_tiles[m][:, sl], op0=ALU.mult, op1=ALU.mult)
                        nc.vector.tensor_add(out_tiles[m][:, sl], out_tiles[m][:, sl], d_t[:])

            # ---------------- layers ----------------
            for l in range(n_layers):
                W = wl[l]
                pv_t = []
                for m in range(NM):
                    pvt = wq_p.tile([128, NPC], F32, name=f"pv{l}_{m}", tag="pvec", bufs=8)
                    nc.sync.dma_start(pvt[:], W["pvec"][m])
                    pv_t.append(pvt)
                fb1_t = wq_p.tile([128, 16], F32, name=f"fb1_{l}", tag="fb1", bufs=2)
                nc.sync.dma_start(fb1_t[:], W["fb1"][:])
                bvr_t = wq_p.tile([128, D], BF16, name=f"bvr_{l}", tag="bvr", bufs=2)
                nc.sync.dma_start(bvr_t[:], W["bvrep"][:])

                # cast residual stream to bf16 for GEMM rhs
                for m in range(NM):
                    nc.gpsimd.tensor_copy(h_bf[m][:], h_a[m][:])

                # ---- Q, K GEMMs (feature-major, bf16) ----
                wq_t = load_w(W["wq"], NM, D, BF16, "wqkvo", 8, f"wq{l}")

                def q_out(m, tci, sl, acc, _pv=pv_t):
                    nc.scalar.activation(q_t[m][:, sl], acc[:], AF.Identity,
                                         bias=_pv[m][:, PC_BQ:PC_BQ + 1])
                gemm_fm_bf16(wq_t, h_bf, NM, q_out)

                wk_t = load_w(W["wk"], NM, D, BF16, "wqkvo", 8, f"wk{l}")

                def k_out(m, tci, sl, acc, _pv=pv_t):
                    nc.scalar.activation(k_t[m][:, sl], acc[:], AF.Identity,
                                         bias=_pv[m][:, PC_BK:PC_BK + 1])
                gemm_fm_bf16(wk_t, h_bf, NM, k_out)

                # ---- V GEMM (token-major: lhsT = h_bf chunk, rhs = wv) ----
                wv_t = load_w(W["wv"], NM, D, BF16, "wqkvo", 8, f"wv{l}")
                for it in range(NV):
                    tsl = slice(it * 125, (it + 1) * 125)
                    acc = ps_big.tile([125, D], F32, name="accv", tag="big", bufs=2)
                    for kc in range(NM):
                        nc.tensor.matmul(acc[:], h_bf[kc][:, tsl], wv_t[kc][:],
                                         start=(kc == 0), stop=(kc == NM - 1))
                    vv = v_t[it][:125].rearrange("p (h c) -> p h c", h=H)[:, :, 0:32]
                    nc.vector.tensor_add(
                        vv, acc[:].rearrange("p (h c) -> p h c", h=H),
                        bvr_t[:125].rearrange("p (h c) -> p h c", h=H))

                # ---- attention per (head, batch) ----
                for h in range(H):
                    mt = h // 4
                    off = (h % 4) * 32
                    if off == 96:
                        qs = sq_p.tile([32, TP], BF16, name="qs", tag="stage", bufs=2)
                        ks = sq_p.tile([32, TP], BF16, name="ks", tag="stage", bufs=2)
                        nc.gpsimd.tensor_copy(qs[:], q_t[mt][96:128, :])
                        nc.gpsimd.tensor_copy(ks[:], k_t[mt][96:128, :])
                        q_src, k_src, soff = qs, ks, 0
                    else:
                        q_src, k_src, soff = q_t[mt], k_t[mt], off
                    for b in range(BC):
                        bsl = slice(b * S, (b + 1) * S)
                        av = ps_scav.tile([33, S], F32, name="av", tag="scav", bufs=4)
                        for c in range(2):
                            ksl = slice(b * S + c * 125, b * S + (c + 1) * 125)
                            sc = ps_scav.tile([125, S], F32, name="sc", tag="scav", bufs=4)
                            nc.tensor.matmul(sc[:], k_src[soff:soff + DK, ksl],
                                             q_src[soff:soff + DK, bsl], start=True, stop=True)
                            es = sq_p.tile([125, S], BF16, name="es", tag="es", bufs=4)
                            nc.scalar.activation(es[:], sc[:], AF.Exp)
                            nc.gpsimd.tensor_mul(es[:], es[:], etb_t[c][:125, :S])
                            nc.tensor.matmul(av[:], v_t[b * 2 + c][:125, h * 33:h * 33 + 33],
                                             es[:], start=(c == 0), stop=(c == 1))
                        nc.vector.tensor_copy(ctx_t[mt][off:off + 32, bsl], av[0:32, :])
                        nc.vector.tensor_copy(den_t[mt][off:off + 1, bsl], av[32:33, :])

                # softmax denominators -> reciprocal -> broadcast -> scale ctx
                for m in range(NM):
                    with nc.allow_low_precision(reason="softmax denominators kept bf16"):
                        nc.vector.reciprocal(den_t[m][:], den_t[m][:])
                    for tci in range(2):
                        sl = slice(tci * TCH, (tci + 1) * TCH)
                        rbm = ps_small.tile([128, TCH], F32, name="rbm", tag="small", bufs=2)
                        nc.tensor.matmul(rbm[:], e128_t[:], den_t[m][:, sl],
                                         start=True, stop=True)
                        nc.vector.tensor_mul(ctx_t[m][:, sl], ctx_t[m][:, sl], rbm[:])
                    # restore denominators to 1.0 for next layer
                    nc.vector.memset(den_t[m][:], 1.0)

                # ---- gate: g = sigmoid(relu(h@g1+gb1) @ g2 + gb2) ----
                g1_t = load_w(W["g1"], NM, D // 2, BF16, "wg1", 6, f"g1{l}")
                relu_t = {}

                def g1_out(m, tci, sl, acc, _pv=pv_t, _rt=relu_t):
                    rt = sq_p.tile([128, TCH], BF16, name="relu", tag="relu", bufs=4)
                    nc.scalar.activation(rt[:], acc[:], AF.Relu,
                                         bias=_pv[m][:, PC_GB1:PC_GB1 + 1])
                    _rt[(m, tci)] = rt
                gemm_fm_bf16(g1_t, h_bf, 2, g1_out)
                g2_t = load_w(W["g2"], 2, 1, BF16, "pvec", 8, f"g2{l}")
                g_chunks = []
                for tci in range(2):
                    sl = slice(tci * TCH, (tci + 1) * TCH)
                    gacc = ps_small.tile([1, TCH], F32, name="gacc", tag="small", bufs=2)
                    for kc in range(2):
                        nc.tensor.matmul(gacc[:], g2_t[kc][:], relu_t[(kc, tci)][:],
                                         start=(kc == 0), stop=(kc == 1))
                    gch = sq_p.tile([1, TCH], F32, name="gch", tag="lns", bufs=5)
                    nc.scalar.activation(gch[:], gacc[:], AF.Sigmoid,
                                         bias=pv_t[0][0:1, PC_GB2:PC_GB2 + 1])
                    g_chunks.append(gch)

                # ---- O GEMM + gated blend: P = g*(o+bo-h) + h ----
                wo_t = load_w(W["wo"], NM, D, BF16, "wqkvo", 8, f"wo{l}")
                gb_ps = []
                for tci in range(2):
                    sl = slice(tci * TCH, (tci + 1) * TCH)
                    gb = ps_small.tile([128, TCH], F32, name="gb", tag="small", bufs=2)
                    nc.tensor.matmul(gb[:], ones1[:], g_chunks[tci][:], start=True, stop=True)
                    gb_ps.append(gb)

                def o_out(m, tci, sl, acc, _pv=pv_t, _gb=gb_ps):
                    nc.vector.scalar_tensor_tensor(
                        P[m][:, sl], acc[:], _pv[m][:, PC_BO:PC_BO + 1], h_a[m][:, sl],
                        op0=ALU.add, op1=ALU.subtract)          # o + bo - h
                    nc.vector.tensor_mul(P[m][:, sl], P[m][:, sl], _gb[tci][:])
                    nc.vector.tensor_add(P[m][:, sl], P[m][:, sl], h_a[m][:, sl])
                gemm_fm_bf16(wo_t, ctx_t, NM, o_out)

                # ---- attn_out = LN_a(P) (in-place); h_a += attn_out; h1 = LN_1(h_a) ----
                emit_ln(P, P, (PC_LAS, PC_LANS, PC_LAB), pv_t)
                for m in range(NM):
                    nc.vector.tensor_add(h_a[m][:], h_a[m][:], P[m][:])
                emit_ln(h_a, h1, (PC_N1S, PC_N1NS, PC_N1B), pv_t)
                for m in range(NM):
                    nc.gpsimd.tensor_copy(h1b[m][:], h1[m][:])

                # ---- FFN: h_a = h1 + gelu(h1@f1+fb1)@f2 + fb2 ; h_next = LN_2 ----
                f1_t = []
                for kc in range(NM):
                    for hh in range(2):
                        wt = wq_p.tile([128, DFF // 2], BF16, name=f"f1{l}_{kc}_{hh}",
                                       tag="wf1", bufs=9)
                        nc.sync.dma_start(wt[:], W["f1"][kc * 128:(kc + 1) * 128,
                                                         hh * (DFF // 2):(hh + 1) * (DFF // 2)])
                        f1_t.append(wt)
                f2_t = load_w(W["f2"], DFF // 128, D, BF16, "wf2", 17, f"f2{l}")
                for tci in range(4):
                    sl = slice(tci * FCH, (tci + 1) * FCH)
                    gelu_t = []
                    for mf in range(DFF // 128):
                        acc = ps_big.tile([128, FCH], F32, name="accf1", tag="big", bufs=2)
                        for kc in range(NM):
                            wt = f1_t[kc * 2 + (mf // 8)]
                            csl = slice((mf % 8) * 128, (mf % 8 + 1) * 128)
                            nc.tensor.matmul(acc[:], wt[:, csl],
                                             h1b[kc][:, sl], start=(kc == 0), stop=(kc == NM - 1))
                        gt = sq_p.tile([128, FCH], BF16, name="gelu", tag="gelu", bufs=17)
                        nc.scalar.activation(gt[:], acc[:], AF.Gelu,
                                             bias=fb1_t[:, mf:mf + 1])
                        gelu_t.append(gt)
                    for m in range(NM):
                        acc = ps_big.tile([128, FCH], F32, name="accf2", tag="big", bufs=2)
                        for kc in range(DFF // 128):
                            nc.tensor.matmul(acc[:], f2_t[kc][:, m * 128:(m + 1) * 128],
                                             gelu_t[kc][:], start=(kc == 0), stop=(kc == DFF // 128 - 1))
                        # h_a[m] = (ff2 + fb2) + h1
                        nc.vector.scalar_tensor_tensor(
                            h_a[m][:, sl], acc[:], pv_t[m][:, PC_FB2:PC_FB2 + 1],
                            h1[m][:, sl], op0=ALU.add, op1=ALU.add)
                emit_ln(h_a, h_b, (PC_N2S, PC_N2NS, PC_N2B), pv_t)
                h_a, h_b = h_b, h_a

            # ---------------- final LN + output ----------------
            emit_ln(h_a, P, (0, 1, 2), fpv_t)
            for m in range(NM):
                nc.sync.dma_start(out_d[m], P[m][:, :T])

    nc.compile()
    return nc


@functools.lru_cache(maxsize=2)
def _get_program(n_layers):
    return _build_program(n_layers)


def _prep_common(w_in, b_in, pe, pos_scale, pos_bias, tbias, layers,
                 final_scale, final_bias, n_layers):
    """host-side constant prep -> dict of common in_map entries."""
    cm = {}
    cm["win"] = np.ascontiguousarray(w_in, np.float32)
    # positional: h = (h0 + b_in + pe) * ps + pb ; ADD = (b_in + pe)*ps + pb
    ps = np.asarray(pos_scale, np.float32).reshape(S, 1)
    pb = np.asarray(pos_bias, np.float32).reshape(S, 1)
    add = ((np.asarray(b_in, np.float32)[None, :] + np.asarray(pe, np.float32)) * ps + pb).T  # [D, S]
    cm["addpos"] = np.ascontiguousarray(add.reshape(NM, 128, S), np.float32)
    cm["psrep"] = np.ascontiguousarray(np.broadcast_to(ps.T, (128, S)), np.float32)
    etbT = np.exp(np.asarray(tbias, np.float32)).T  # [sk, sq]
    etb = np.zeros((2, 128, 256), BF16NP)
    for c in range(2):
        etb[c, :125, :S] = etbT[c * 125:(c + 1) * 125, :].astype(BF16NP)
    cm["etb"] = etb
    e128 = np.zeros((128, 128), BF16NP)
    for p in range(128):
        e128[(p // 32) * 32, p] = BF16NP(1.0)
    cm["e128"] = e128
    fpv = np.zeros((NM, 128, 4), np.float32)
    fs = np.asarray(final_scale, np.float32).reshape(NM, 128)
    fb = np.asarray(final_bias, np.float32).reshape(NM, 128)
    fpv[:, :, 0] = fs
    fpv[:, :, 1] = -fs
    fpv[:, :, 2] = fb
    cm["fpvec"] = fpv

    scale = 1.0 / math.sqrt(DK)
    ly = {k: np.asarray(v) for k, v in layers.items()}
    for l in range(n_layers):
        cm[f"L{l}_wq"] = (ly["wq"][l] * scale).astype(BF16NP)
        cm[f"L{l}_wk"] = ly["wk"][l].astype(BF16NP)
        cm[f"L{l}_wv"] = ly["wv"][l].astype(BF16NP)
        cm[f"L{l}_wo"] = ly["wo"][l].astype(BF16NP)
        cm[f"L{l}_g1"] = ly["g1"][l].astype(BF16NP)
        cm[f"L{l}_g2"] = ly["g2"][l].astype(BF16NP)
        cm[f"L{l}_f1"] = ly["f1"][l].astype(BF16NP)
        cm[f"L{l}_f2"] = ly["f2"][l].astype(BF16NP)
        pvec = np.zeros((NM, 128, NPC), np.float32)
        pvec[:, :, PC_BQ] = (ly["bq"][l] * scale).reshape(NM, 128)
        pvec[:, :, PC_BK] = ly["bk"][l].reshape(NM, 128)
        pvec[:, :, PC_BO] = ly["bo"][l].reshape(NM, 128)
        pvec[:, :, PC_FB2] = ly["fb2"][l].reshape(NM, 128)
        for (cs, cns, cb), nm_ in [((PC_LAS, PC_LANS, PC_LAB), ("lna_s", "lna_b")),
                                   ((PC_N1S, PC_N1NS, PC_N1B), ("n1s", "n1b")),
                                   ((PC_N2S, PC_N2NS, PC_N2B), ("n2s", "n2b"))]:
            sv = ly[nm_[0]][l].reshape(NM, 128)
            bv_ = ly[nm_[1]][l].reshape(NM, 128)
            pvec[:, :, cs] = sv
            pvec[:, :, cns] = -sv
            pvec[:, :, cb] = bv_
        gb1 = ly["gb1"][l].reshape(2, 128)
        pvec[0:2, :, PC_GB1] = gb1
        pvec[0, 0, PC_GB2] = float(ly["gb2"][l].reshape(-1)[0])
        cm[f"L{l}_pvec"] = pvec
        cm[f"L{l}_fb1"] = np.ascontiguousarray(ly["fb1"][l].reshape(16, 128).T, np.float32)
        cm[f"L{l}_bvrep"] = np.broadcast_to(
            ly["bv"][l].astype(BF16NP), (128, D)).copy()
    return cm


def make_in_maps(x, **consts):
    """build the 8 per-core input maps (full inputs -> shards)."""
    n_layers = N_LAYERS
    cm = _prep_common(n_layers=n_layers, **consts)
    x = np.asarray(x, np.float32)
    in_maps = []
    for i in range(NCORES):
        shard = x[i * BC:(i + 1) * BC]                    # [4, 250, 64]
        xT = np.zeros((IN, TP), np.float32)
        xT[:, :T] = shard.reshape(T, IN).T
        m = dict(cm)
        m["xT"] = xT
        in_maps.append(m)
    return in_maps


def kernel(x, w_in, b_in, pe, pos_scale, pos_bias, tbias, layers,
           final_scale, final_bias):
    nc = _get_program(N_LAYERS)
    in_maps = make_in_maps(
        x, w_in=w_in, b_in=b_in, pe=pe, pos_scale=pos_scale,
        pos_bias=pos_bias, tbias=tbias, layers=layers,
        final_scale=final_scale, final_bias=final_bias)
    res = run_bass_kernel_spmd(nc, in_maps, core_ids=list(range(NCORES)))
    outs = []
    for i in range(NCORES):
        o = np.asarray(res.results[i]["out"])             # [4, 128, 1000]
        o = o.reshape(D, BC, S).transpose(1, 2, 0)        # [4, 250, 512]
        outs.append(o)
    return np.concatenate(outs, axis=0).astype(np.float32)


# revision 11
# speedup vs baseline: 28.6787x; 28.6787x over previous
"""Trainium2 Bass kernel for nn_EnhancedCryptoTransformer (8-layer post-LN
transformer, B=32 S=250 D=512 H=16 DFF=2048, gated attention blend, causal
exponential time-decay additive bias).

Sharding: pure data-parallel over batch - 4 sequences per NeuronCore, no
collectives.  Per-core activations are kept feature-major [D, T] (T=1000
tokens) so every GEMM uses natural-layout weights as the PE stationary
operand.  Attention computes scores^T per (batch, head) so softmax
normalizers fall out of the AV matmul via a ones-column interleaved into the
token-major V tiles.  LayerNorm statistics use PE ones-matmul partition
reductions; per-token stats are broadcast back across partitions with
indicator-matrix matmuls.
"""

import functools
import math
import os
import sys

sys.path.insert(0, "/opt/trn_rl_repo")

import numpy as np
import ml_dtypes

import concourse.bacc as bacc
import concourse.bass as bass
import concourse.mybir as mybir
import concourse.tile as tile
from concourse.bass_utils import run_bass_kernel_spmd

F32 = mybir.dt.float32
F32R = mybir.dt.float32r
BF16 = mybir.dt.bfloat16
AF = mybir.ActivationFunctionType
ALU = mybir.AluOpType
BF16NP = ml_dtypes.bfloat16

B, S, D, H, L, DFF, IN = 32, 250, 512, 16, 8, 2048, 64
DK = D // H                      # 32
NCORES = 8
BC = B // NCORES                 # 4 sequences per core
T = BC * S                       # 1000 tokens per core
TP = 1008                        # padded free size
TCH = TP // 2                    # 504 t-chunk for GEMMs / LN
FCH = TP // 4                    # 252 t-chunk for FFN
NM = D // 128                    # 4 partition tiles per [D, T] tensor
NV = T // 125                    # 8 token-major v tiles of 125 tokens
EPS = 1e-5

N_LAYERS = int(os.environ.get("KERNEL_LAYERS", L))

# pvec column map (per-partition scalars, packed [4, 128, 16] per layer)
PC_BQ, PC_BK, PC_BO, PC_FB2 = 0, 1, 2, 3
PC_LAS, PC_LANS, PC_LAB = 4, 5, 6
PC_N1S, PC_N1NS, PC_N1B = 7, 8, 9
PC_N2S, PC_N2NS, PC_N2B = 10, 11, 12
PC_GB1, PC_GB2 = 13, 14
NPC = 16


def _build_program(n_layers: int):
    nc = bacc.Bacc("TRN2", target_bir_lowering=False, debug=False)

    # ---------------- DRAM I/O ----------------
    xT_d = nc.dram_tensor("xT", [IN, TP], F32, kind="ExternalInput")
    win_d = nc.dram_tensor("win", [IN, D], F32, kind="ExternalInput")
    add_d = nc.dram_tensor("addpos", [NM, 128, S], F32, kind="ExternalInput")
    ps_d = nc.dram_tensor("psrep", [128, S], F32, kind="ExternalInput")
    etb_d = nc.dram_tensor("etb", [2, 128, 256], BF16, kind="ExternalInput")
    e128_d = nc.dram_tensor("e128", [128, 128], BF16, kind="ExternalInput")
    fpvec_d = nc.dram_tensor("fpvec", [NM, 128, 4], F32, kind="ExternalInput")
    out_d = nc.dram_tensor("out", [NM, 128, T], F32, kind="ExternalOutput")

    wl = []
    for l in range(n_layers):
        wl.append(dict(
            wq=nc.dram_tensor(f"L{l}_wq", [D, D], BF16, kind="ExternalInput"),
            wk=nc.dram_tensor(f"L{l}_wk", [D, D], BF16, kind="ExternalInput"),
            wv=nc.dram_tensor(f"L{l}_wv", [D, D], BF16, kind="ExternalInput"),
            wo=nc.dram_tensor(f"L{l}_wo", [D, D], BF16, kind="ExternalInput"),
            g1=nc.dram_tensor(f"L{l}_g1", [D, D // 2], BF16, kind="ExternalInput"),
            g2=nc.dram_tensor(f"L{l}_g2", [D // 2, 1], BF16, kind="ExternalInput"),
            f1=nc.dram_tensor(f"L{l}_f1", [D, DFF], BF16, kind="ExternalInput"),
            f2=nc.dram_tensor(f"L{l}_f2", [DFF, D], BF16, kind="ExternalInput"),
            pvec=nc.dram_tensor(f"L{l}_pvec", [NM, 128, NPC], F32, kind="ExternalInput"),
            fb1=nc.dram_tensor(f"L{l}_fb1", [128, 16], F32, kind="ExternalInput"),
            bvrep=nc.dram_tensor(f"L{l}_bvrep", [128, D], BF16, kind="ExternalInput"),
        ))

    with tile.TileContext(nc) as tc:
        import contextlib
        ctx = contextlib.ExitStack()
        with ctx:
            # ---------------- pools ----------------
            per = ctx.enter_context(tc.tile_pool(name="per", bufs=1))          # persistent
            wq_p = ctx.enter_context(tc.tile_pool(name="wq_p", bufs=1))        # weights (tags manage slots)
            sq_p = ctx.enter_context(tc.tile_pool(name="sq_p", bufs=1))
            ps_big = ctx.enter_context(tc.tile_pool(name="ps_big", bufs=1, space="PSUM"))
            ps_scav = ctx.enter_context(tc.tile_pool(name="ps_scav", bufs=1, space="PSUM"))
            ps_small = ctx.enter_context(tc.tile_pool(name="ps_small", bufs=1, space="PSUM"))

            def ptile(shape, dt, name, pool=per, tag=None, bufs=1):
                return pool.tile(shape, dt, name=name, tag=tag or name, bufs=bufs)

            # ---------------- persistent big buffers ----------------
            h_a = [ptile([128, TP], F32, f"h_a{m}") for m in range(NM)]
            h_b = [ptile([128, TP], F32, f"h_b{m}") for m in range(NM)]
            P = [ptile([128, TP], F32, f"P{m}") for m in range(NM)]       # blend / final-LN out
            h1 = [ptile([128, TP], F32, f"h1{m}") for m in range(NM)]
            h_bf = [ptile([128, TP], BF16, f"hbf{m}") for m in range(NM)]
            h1b = h_bf  # reused: h_bf is dead once the gate GEMM has consumed it
            q_t = [ptile([128, TP], BF16, f"q{m}") for m in range(NM)]
            k_t = [ptile([128, TP], BF16, f"k{m}") for m in range(NM)]
            ctx_t = [ptile([128, TP], BF16, f"ctx{m}") for m in range(NM)]
            v_t = [ptile([128, H * 33], BF16, f"v{m}") for m in range(NV)]   # [125 tokens, 16*(32+1)]
            den_t = [ptile([128, TP], BF16, f"den{m}") for m in range(NM)]

            # ---------------- constants ----------------
            ones128 = ptile([128, 1], F32, "ones128")
            nc.vector.memset(ones128[:], 1.0)
            ones1 = ptile([1, 128], F32, "ones1")
            nc.vector.memset(ones1[:], 1.0)
            ones128b = ptile([128, 1], BF16, "ones128b")
            nc.vector.memset(ones128b[:], 1.0)
            eps_t = ptile([1, 1], F32, "eps_t")
            nc.vector.memset(eps_t[:], EPS)
            e128_t = ptile([128, 128], BF16, "e128_t")
            nc.sync.dma_start(e128_t[:], e128_d[:])
            etb_t = [ptile([128, 256], BF16, f"etb{c}") for c in range(2)]
            for c in range(2):
                nc.sync.dma_start(etb_t[c][:], etb_d[c])
            fpv_t = [ptile([128, 4], F32, f"fpv{m}") for m in range(NM)]
            for m in range(NM):
                nc.sync.dma_start(fpv_t[m][:], fpvec_d[m])

            # init: v ones-columns, denominators, pad columns of h_a
            for i in range(NV):
                nc.vector.memset(v_t[i][:125].rearrange("p (h c) -> p h c", h=H)[:, :, 32:33], 1.0)
            for m in range(NM):
                nc.vector.memset(den_t[m][:], 1.0)
                nc.vector.memset(ctx_t[m][:, T:TP], 0.0)
                nc.vector.memset(h_a[m][:, T:TP], 0.0)
                nc.vector.memset(h_b[m][:, T:TP], 0.0)
                nc.vector.memset(h1[m][:, T:TP], 0.0)
                nc.vector.memset(P[m][:, T:TP], 0.0)

            def r32(ap):
                return ap.bitcast(F32R)

            # ---------------- input projection + positional ----------------
            xT_t = [sq_p.tile([64, TCH], F32, name=f"xT{i}", tag="sq", bufs=3) for i in range(2)]
            for i in range(2):
                nc.sync.dma_start(xT_t[i][:], xT_d[:, i * TCH:(i + 1) * TCH])
            win_t = [wq_p.tile([IN, 128], F32, name=f"win{m}", tag="wqkvo", bufs=8) for m in range(NM)]
            for m in range(NM):
                nc.sync.dma_start(win_t[m][:], win_d[:, m * 128:(m + 1) * 128])
            for m in range(NM):
                for tci in range(2):
                    acc = ps_big.tile([128, TCH], F32, name="accin", tag="big", bufs=2)
                    nc.tensor.matmul(acc[:], win_t[m][:], xT_t[tci][:], start=True, stop=True)
                    nc.scalar.copy(h_a[m][:, tci * TCH:(tci + 1) * TCH], acc[:])
            ps_t = sq_p.tile([128, S], F32, name="ps_t", tag="sq", bufs=3)
            nc.sync.dma_start(ps_t[:], ps_d[:])
            for m in range(NM):
                add_t = sq_p.tile([128, S], F32, name=f"add{m}", tag="sq", bufs=3)
                nc.sync.dma_start(add_t[:], add_d[m])
                for b in range(BC):
                    sl = slice(b * S, (b + 1) * S)
                    nc.vector.tensor_mul(h_a[m][:, sl], h_a[m][:, sl], ps_t[:])
                    nc.vector.tensor_add(h_a[m][:, sl], h_a[m][:, sl], add_t[:])

            # ---------------- helpers ----------------
            def load_w(dram, kparts, ncols, dt, tag, bufs, name):
                """load [K, N] dram weight into kparts tiles of [128, ncols]."""
                ts = []
                for kc in range(kparts):
                    wt = wq_p.tile([128, ncols], dt, name=f"{name}_{kc}", tag=tag, bufs=bufs)
                    nc.sync.dma_start(wt[:], dram[kc * 128:(kc + 1) * 128, :])
                    ts.append(wt)
                return ts

            def gemm_fm_bf16(w_tiles, rhs_tiles, nm_out, out_cb, kparts=NM, tch=TCH):
                """feature-major GEMM: out[m*128:(m+1)*128, tchunk] tiles via PSUM.
                w_tiles: kparts tiles [128, nm_out*128] bf16 (lhsT slices taken per m)
                rhs_tiles: kparts activation tiles [128, TP] bf16
                out_cb(m, tci, acc_psum): consume psum [128, tch]"""
                nchunks = TP // tch
                for m in range(nm_out):
                    for tci in range(nchunks):
                        sl = slice(tci * tch, (tci + 1) * tch)
                        acc = ps_big.tile([128, tch], F32, name="acc", tag="big", bufs=2)
                        for kc in range(kparts):
                            nc.tensor.matmul(
                                acc[:], w_tiles[kc][:, m * 128:(m + 1) * 128],
                                rhs_tiles[kc][:, sl],
                                start=(kc == 0), stop=(kc == kparts - 1))
                        out_cb(m, tci, sl, acc)

            def emit_ln(x_tiles, out_tiles, pv_idx, pv_tiles=None, bf_mirror=None):
                """feature-major LayerNorm over D: out = (x-m)/sd*gamma+beta.
                pv_idx = (s_col, negs_col, b_col); pv_tiles: list of [128, NPC] tiles."""
                s_col, ns_col, b_col = pv_idx
                for tci in range(2):
                    sl = slice(tci * TCH, (tci + 1) * TCH)
                    ssum = ps_small.tile([1, TCH], F32, name="ssum", tag="small", bufs=2)
                    ssq = ps_small.tile([1, TCH], F32, name="ssq", tag="small", bufs=2)
                    for m in range(NM):
                        xb = sq_p.tile([128, TCH], BF16, name="xb", tag="sqb", bufs=6)
                        nc.gpsimd.tensor_copy(xb[:], x_tiles[m][:, sl])
                        sqb = sq_p.tile([128, TCH], BF16, name="sqb", tag="sqb", bufs=6)
                        nc.gpsimd.tensor_mul(sqb[:], xb[:], xb[:])
                        nc.tensor.matmul(ssum[:], ones128b[:], xb[:],
                                         start=(m == 0), stop=(m == NM - 1))
                        nc.tensor.matmul(ssq[:], ones128b[:], sqb[:],
                                         start=(m == 0), stop=(m == NM - 1))
                    a0 = sq_p.tile([1, TCH], F32, name="a0", tag="lns", bufs=5)
                    a1 = sq_p.tile([1, TCH], F32, name="a1", tag="lns", bufs=5)
                    v1 = sq_p.tile([1, TCH], F32, name="v1", tag="lns", bufs=5)
                    nc.scalar.activation(a0[:], ssum[:], AF.Copy, scale=1.0 / D)
                    nc.scalar.activation(a1[:], ssq[:], AF.Copy, scale=1.0 / D)
                    nc.vector.tensor_mul(v1[:], a0[:], a0[:])
                    nc.vector.tensor_sub(v1[:], a1[:], v1[:])
                    nc.scalar.activation(v1[:], v1[:], AF.Sqrt, bias=eps_t[:])
                    nc.vector.reciprocal(v1[:], v1[:])           # r = 1/sd
                    nc.vector.tensor_mul(a0[:], a0[:], v1[:])    # mr = mean * r
                    rb = ps_small.tile([128, TCH], F32, name="rb", tag="small", bufs=2)
                    mrb = ps_small.tile([128, TCH], F32, name="mrb", tag="small", bufs=2)
                    nc.tensor.matmul(rb[:], ones1[:], v1[:], start=True, stop=True)
                    nc.tensor.matmul(mrb[:], ones1[:], a0[:], start=True, stop=True)
                    for m in range(NM):
                        pv = pv_tiles[m]
                        d_t = sq_p.tile([128, TCH], F32, name="d_t", tag="d_t", bufs=2)
                        nc.scalar.activation(d_t[:], mrb[:], AF.Identity,
                                             bias=pv[:, b_col:b_col + 1],
                                             scale=pv[:, ns_col:ns_col + 1])
                        nc.vector.scalar_tensor_tensor(
                            out_tiles[m][:, sl], rb[:], pv[:, s_col:s_col + 1],
                            x_tiles[m][:, sl], op0=ALU.mult, op1=ALU.mult)
                        nc.vector.tensor_add(out_tiles[m][:, sl], out_tiles[m][:, sl], d_t[:])
                        if bf_mirror is not None:
                            nc.gpsimd.tensor_copy(bf_mirror[m][:, sl], out_tiles[m][:, sl])

            # ---------------- layers ----------------
            for l in range(n_layers):
                W = wl[l]
                pv_t = []
                for m in range(NM):
                    pvt = wq_p.tile([128, NPC], F32, name=f"pv{l}_{m}", tag="pvec", bufs=8)
                    nc.sync.dma_start(pvt[:], W["pvec"][m])
                    pv_t.append(pvt)
                fb1_t = wq_p.tile([128, 16], F32, name=f"fb1_{l}", tag="fb1", bufs=2)
                nc.sync.dma_start(fb1_t[:], W["fb1"][:])
                bvr_t = wq_p.tile([128, D], BF16, name=f"bvr_{l}", tag="bvr", bufs=2)
                nc.sync.dma_start(bvr_t[:], W["bvrep"][:])

                if l == 0:
                    for m in range(NM):
                        nc.gpsimd.tensor_copy(h_bf[m][:], h_a[m][:])

                # ---- Q, K GEMMs (feature-major, bf16) ----
                wq_t = load_w(W["wq"], NM, D, BF16, "wqkvo", 8, f"wq{l}")

                def q_out(m, tci, sl, acc, _pv=pv_t):
                    nc.scalar.activation(q_t[m][:, sl], acc[:], AF.Identity,
                                         bias=_pv[m][:, PC_BQ:PC_BQ + 1])
                gemm_fm_bf16(wq_t, h_bf, NM, q_out)

                wk_t = load_w(W["wk"], NM, D, BF16, "wqkvo", 8, f"wk{l}")

                def k_out(m, tci, sl, acc, _pv=pv_t):
                    nc.scalar.activation(k_t[m][:, sl], acc[:], AF.Identity,
                                         bias=_pv[m][:, PC_BK:PC_BK + 1])
                gemm_fm_bf16(wk_t, h_bf, NM, k_out)

                # ---- V GEMM (token-major: lhsT = h_bf chunk, rhs = wv) ----
                wv_t = load_w(W["wv"], NM, D, BF16, "wqkvo", 8, f"wv{l}")
                for it in range(NV):
                    tsl = slice(it * 125, (it + 1) * 125)
                    acc = ps_big.tile([125, D], F32, name="accv", tag="big", bufs=2)
                    for kc in range(NM):
                        nc.tensor.matmul(acc[:], h_bf[kc][:, tsl], wv_t[kc][:],
                                         start=(kc == 0), stop=(kc == NM - 1))
                    vv = v_t[it][:125].rearrange("p (h c) -> p h c", h=H)[:, :, 0:32]
                    nc.vector.tensor_add(
                        vv, acc[:].rearrange("p (h c) -> p h c", h=H),
                        bvr_t[:125].rearrange("p (h c) -> p h c", h=H))

                # ---- attention per (head, batch) ----
                for h in range(H):
                    mt = h // 4
                    off = (h % 4) * 32
                    if off == 96:
                        qs = sq_p.tile([32, TP], BF16, name="qs", tag="stage", bufs=2)
                        ks = sq_p.tile([32, TP], BF16, name="ks", tag="stage", bufs=2)
                        nc.gpsimd.tensor_copy(qs[:], q_t[mt][96:128, :])
                        nc.gpsimd.tensor_copy(ks[:], k_t[mt][96:128, :])
                        q_src, k_src, soff = qs, ks, 0
                    else:
                        q_src, k_src, soff = q_t[mt], k_t[mt], off
                    for b in range(BC):
                        bsl = slice(b * S, (b + 1) * S)
                        av = ps_small.tile([33, S], F32, name="av", tag="small", bufs=2)
                        for c in range(2):
                            ksl = slice(b * S + c * 125, b * S + (c + 1) * 125)
                            sc = ps_scav.tile([125, S], F32, name="sc", tag="sc", bufs=4)
                            nc.tensor.matmul(sc[:], k_src[soff:soff + DK, ksl],
                                             q_src[soff:soff + DK, bsl], start=True, stop=True)
                            es = sq_p.tile([125, S], BF16, name="es", tag="es", bufs=6)
                            nc.scalar.activation(es[:], sc[:], AF.Exp)
                            nc.vector.tensor_mul(es[:], es[:], etb_t[c][:125, :S])
                            nc.tensor.matmul(av[:], v_t[b * 2 + c][:125, h * 33:h * 33 + 33],
                                             es[:], start=(c == 0), stop=(c == 1))
                        nc.vector.tensor_copy(ctx_t[mt][off:off + 32, bsl], av[0:32, :])
                        nc.vector.tensor_copy(den_t[mt][off:off + 1, bsl], av[32:33, :])

                # softmax denominators -> reciprocal -> broadcast -> scale ctx
                for m in range(NM):
                    with nc.allow_low_precision(reason="softmax denominators kept bf16"):
                        nc.vector.reciprocal(den_t[m][:], den_t[m][:])
                    for tci in range(2):
                        sl = slice(tci * TCH, (tci + 1) * TCH)
                        rbm = ps_small.tile([128, TCH], F32, name="rbm", tag="small", bufs=2)
                        nc.tensor.matmul(rbm[:], e128_t[:], den_t[m][:, sl],
                                         start=True, stop=True)
                        nc.vector.tensor_mul(ctx_t[m][:, sl], ctx_t[m][:, sl], rbm[:])
                    # restore denominators to 1.0 for next layer
                    nc.vector.memset(den_t[m][:], 1.0)

                # ---- gate: g = sigmoid(relu(h@g1+gb1) @ g2 + gb2) ----
                g1_t = load_w(W["g1"], NM, D // 2, BF16, "wg1", 6, f"g1{l}")
                relu_t = {}

                def g1_out(m, tci, sl, acc, _pv=pv_t, _rt=relu_t):
                    rt = sq_p.tile([128, TCH], BF16, name="relu", tag="relu", bufs=4)
                    nc.scalar.activation(rt[:], acc[:], AF.Relu,
                                         bias=_pv[m][:, PC_GB1:PC_GB1 + 1])
                    _rt[(m, tci)] = rt
                gemm_fm_bf16(g1_t, h_bf, 2, g1_out)
                g2_t = load_w(W["g2"], 2, 1, BF16, "pvec", 8, f"g2{l}")
                g_chunks = []
                for tci in range(2):
                    sl = slice(tci * TCH, (tci + 1) * TCH)
                    gacc = ps_small.tile([1, TCH], F32, name="gacc", tag="small", bufs=2)
                    for kc in range(2):
                        nc.tensor.matmul(gacc[:], g2_t[kc][:], relu_t[(kc, tci)][:],
                                         start=(kc == 0), stop=(kc == 1))
                    gch = sq_p.tile([1, TCH], F32, name="gch", tag="lns", bufs=5)
                    nc.scalar.activation(gch[:], gacc[:], AF.Sigmoid,
                                         bias=pv_t[0][0:1, PC_GB2:PC_GB2 + 1])
                    g_chunks.append(gch)

                # ---- O GEMM + gated blend: P = g*(o+bo-h) + h ----
                wo_t = load_w(W["wo"], NM, D, BF16, "wqkvo", 8, f"wo{l}")
                gb_ps = []
                for tci in range(2):
                    sl = slice(tci * TCH, (tci + 1) * TCH)
                    gb = ps_small.tile([128, TCH], F32, name="gb", tag="small", bufs=2)
                    nc.tensor.matmul(gb[:], ones1[:], g_chunks[tci][:], start=True, stop=True)
                    gb_ps.append(gb)

                def o_out(m, tci, sl, acc, _pv=pv_t, _gb=gb_ps):
                    nc.vector.scalar_tensor_tensor(
                        P[m][:, sl], acc[:], _pv[m][:, PC_BO:PC_BO + 1], h_a[m][:, sl],
                        op0=ALU.add, op1=ALU.subtract)          # o + bo - h
                    nc.vector.tensor_mul(P[m][:, sl], P[m][:, sl], _gb[tci][:])
                    nc.vector.tensor_add(P[m][:, sl], P[m][:, sl], h_a[m][:, sl])
                gemm_fm_bf16(wo_t, ctx_t, NM, o_out)

                # ---- attn_out = LN_a(P) (in-place); h_a += attn_out; h1 = LN_1(h_a) ----
                emit_ln(P, P, (PC_LAS, PC_LANS, PC_LAB), pv_t)
                for m in range(NM):
                    nc.vector.tensor_add(h_a[m][:], h_a[m][:], P[m][:])
                emit_ln(h_a, h1, (PC_N1S, PC_N1NS, PC_N1B), pv_t, bf_mirror=h1b)

                # ---- FFN: h_a = h1 + gelu(h1@f1+fb1)@f2 + fb2 ; h_next = LN_2 ----
                f1_t = []
                for kc in range(NM):
                    for hh in range(2):
                        wt = wq_p.tile([128, DFF // 2], BF16, name=f"f1{l}_{kc}_{hh}",
                                       tag="wf1", bufs=9)
                        nc.sync.dma_start(wt[:], W["f1"][kc * 128:(kc + 1) * 128,
                                                         hh * (DFF // 2):(hh + 1) * (DFF // 2)])
                        f1_t.append(wt)
                f2_t = load_w(W["f2"], DFF // 128, D, BF16, "wf2", 17, f"f2{l}")
                for tci in range(4):
                    sl = slice(tci * FCH, (tci + 1) * FCH)
                    gelu_t = []
                    for mf in range(DFF // 128):
                        acc = ps_big.tile([128, FCH], F32, name="accf1", tag="big", bufs=2)
                        for kc in range(NM):
                            wt = f1_t[kc * 2 + (mf // 8)]
                            csl = slice((mf % 8) * 128, (mf % 8 + 1) * 128)
                            nc.tensor.matmul(acc[:], wt[:, csl],
                                             h1b[kc][:, sl], start=(kc == 0), stop=(kc == NM - 1))
                        gt = sq_p.tile([128, FCH], BF16, name="gelu", tag="gelu", bufs=17)
                        nc.scalar.activation(gt[:], acc[:], AF.Gelu,
                                             bias=fb1_t[:, mf:mf + 1])
                        gelu_t.append(gt)
                    for m in range(NM):
                        acc = ps_big.tile([128, FCH], F32, name="accf2", tag="big", bufs=2)
                        for kc in range(DFF // 128):
                            nc.tensor.matmul(acc[:], f2_t[kc][:, m * 128:(m + 1) * 128],
                                             gelu_t[kc][:], start=(kc == 0), stop=(kc == DFF // 128 - 1))
                        # h_a[m] = (ff2 + fb2) + h1
                        nc.vector.scalar_tensor_tensor(
                            h_a[m][:, sl], acc[:], pv_t[m][:, PC_FB2:PC_FB2 + 1],
                            h1[m][:, sl], op0=ALU.add, op1=ALU.add)
                emit_ln(h_a, h_b, (PC_N2S, PC_N2NS, PC_N2B), pv_t, bf_mirror=h_bf)
                h_a, h_b = h_b, h_a

            # ---------------- final LN + output ----------------
            emit_ln(h_a, P, (0, 1, 2), fpv_t)
            for m in range(NM):
                nc.sync.dma_start(out_d[m], P[m][:, :T])

    nc.compile()
    return nc


@functools.lru_cache(maxsize=2)
def _get_program(n_layers):
    return _build_program(n_layers)


def _prep_common(w_in, b_in, pe, pos_scale, pos_bias, tbias, layers,
                 final_scale, final_bias, n_layers):
    """host-side constant prep -> dict of common in_map entries."""
    cm = {}
    cm["win"] = np.ascontiguousarray(w_in, np.float32)
    # positional: h = (h0 + b_in + pe) * ps + pb ; ADD = (b_in + pe)*ps + pb
    ps = np.asarray(pos_scale, np.float32).reshape(S, 1)
    pb = np.asarray(pos_bias, np.float32).reshape(S, 1)
    add = ((np.asarray(b_in, np.float32)[None, :] + np.asarray(pe, np.float32)) * ps + pb).T  # [D, S]
    cm["addpos"] = np.ascontiguousarray(add.reshape(NM, 128, S), np.float32)
    cm["psrep"] = np.ascontiguousarray(np.broadcast_to(ps.T, (128, S)), np.float32)
    etbT = np.exp(np.asarray(tbias, np.float32)).T  # [sk, sq]
    etb = np.zeros((2, 128, 256), BF16NP)
    for c in range(2):
        etb[c, :125, :S] = etbT[c * 125:(c + 1) * 125, :].astype(BF16NP)
    cm["etb"] = etb
    e128 = np.zeros((128, 128), BF16NP)
    for p in range(128):
        e128[(p // 32) * 32, p] = BF16NP(1.0)
    cm["e128"] = e128
    fpv = np.zeros((NM, 128, 4), np.float32)
    fs = np.asarray(final_scale, np.float32).reshape(NM, 128)
    fb = np.asarray(final_bias, np.float32).reshape(NM, 128)
    fpv[:, :, 0] = fs
    fpv[:, :, 1] = -fs
    fpv[:, :, 2] = fb
    cm["fpvec"] = fpv

    scale = 1.0 / math.sqrt(DK)
    ly = {k: np.asarray(v) for k, v in layers.items()}
    for l in range(n_layers):
        cm[f"L{l}_wq"] = (ly["wq"][l] * scale).astype(BF16NP)
        cm[f"L{l}_wk"] = ly["wk"][l].astype(BF16NP)
        cm[f"L{l}_wv"] = ly["wv"][l].astype(BF16NP)
        cm[f"L{l}_wo"] = ly["wo"][l].astype(BF16NP)
        cm[f"L{l}_g1"] = ly["g1"][l].astype(BF16NP)
        cm[f"L{l}_g2"] = ly["g2"][l].astype(BF16NP)
        cm[f"L{l}_f1"] = ly["f1"][l].astype(BF16NP)
        cm[f"L{l}_f2"] = ly["f2"][l].astype(BF16NP)
        pvec = np.zeros((NM, 128, NPC), np.float32)
        pvec[:, :, PC_BQ] = (ly["bq"][l] * scale).reshape(NM, 128)
        pvec[:, :, PC_BK] = ly["bk"][l].reshape(NM, 128)
        pvec[:, :, PC_BO] = ly["bo"][l].reshape(NM, 128)
        pvec[:, :, PC_FB2] = ly["fb2"][l].reshape(NM, 128)
        for (cs, cns, cb), nm_ in [((PC_LAS, PC_LANS, PC_LAB), ("lna_s", "lna_b")),
                                   ((PC_N1S, PC_N1NS, PC_N1B), ("n1s", "n1b")),
                                   ((PC_N2S, PC_N2NS, PC_N2B), ("n2s", "n2b"))]:
            sv = ly[nm_[0]][l].reshape(NM, 128)
            bv_ = ly[nm_[1]][l].reshape(NM, 128)
            pvec[:, :, cs] = sv
            pvec[:, :, cns] = -sv
            pvec[:, :, cb] = bv_
        gb1 = ly["gb1"][l].reshape(2, 128)
        pvec[0:2, :, PC_GB1] = gb1
        pvec[0, 0, PC_GB2] = float(ly["gb2"][l].reshape(-1)[0])
        cm[f"L{l}_pvec"] = pvec
        cm[f"L{l}_fb1"] = np.ascontiguousarray(ly["fb1"][l].reshape(16, 128).T, np.float32)
        cm[f"L{l}_bvrep"] = np.broadcast_to(
            ly["bv"][l].astype(BF16NP), (128, D)).copy()
    return cm


def make_in_maps(x, **consts):
    """build the 8 per-core input maps (full inputs -> shards)."""
    n_layers = N_LAYERS
    cm = _prep_common(n_layers=n_layers, **consts)
    x = np.asarray(x, np.float32)
    in_maps = []
    for i in range(NCORES):
        shard = x[i * BC:(i + 1) * BC]                    # [4, 250, 64]
        xT = np.zeros((IN, TP), np.float32)
        xT[:, :T] = shard.reshape(T, IN).T
        m = dict(cm)
        m["xT"] = xT
        in_maps.append(m)
    return in_maps


def kernel(x, w_in, b_in, pe, pos_scale, pos_bias, tbias, layers,
           final_scale, final_bias):
    nc = _get_program(N_LAYERS)
    in_maps = make_in_maps(
        x, w_in=w_in, b_in=b_in, pe=pe, pos_scale=pos_scale,
        pos_bias=pos_bias, tbias=tbias, layers=layers,
        final_scale=final_scale, final_bias=final_bias)
    res = run_bass_kernel_spmd(nc, in_maps, core_ids=list(range(NCORES)))
    outs = []
    for i in range(NCORES):
        o = np.asarray(res.results[i]["out"])             # [4, 128, 1000]
        o = o.reshape(D, BC, S).transpose(1, 2, 0)        # [4, 250, 512]
        outs.append(o)
    return np.concatenate(outs, axis=0).astype(np.float32)


# revision 27
# speedup vs baseline: 49.6603x; 1.7316x over previous
"""Trainium2 Bass kernel for nn_EnhancedCryptoTransformer (8-layer post-LN
transformer, B=32 S=250 D=512 H=16 DFF=2048, gated attention blend, causal
exponential time-decay additive bias).

Sharding: pure data-parallel over batch - 4 sequences per NeuronCore, no
collectives.  Per-core activations are kept feature-major [D, T] (T=1000
tokens) so every GEMM uses natural-layout weights as the PE stationary
operand.  Attention computes scores^T per (batch, head) so softmax
normalizers fall out of the AV matmul via a ones-column interleaved into the
token-major V tiles.  LayerNorm statistics use PE ones-matmul partition
reductions; per-token stats are broadcast back across partitions with
indicator-matrix matmuls.
"""

import functools
import math
import os
import sys

sys.path.insert(0, "/opt/trn_rl_repo")

import numpy as np
import ml_dtypes

import concourse.bacc as bacc
import concourse.bass as bass
import concourse.mybir as mybir
import concourse.tile as tile
from concourse.bass_utils import run_bass_kernel_spmd

F32 = mybir.dt.float32
F32R = mybir.dt.float32r
F16 = mybir.dt.float16
BF16 = mybir.dt.bfloat16
AF = mybir.ActivationFunctionType
ALU = mybir.AluOpType
BF16NP = ml_dtypes.bfloat16

B, S, D, H, L, DFF, IN = 32, 250, 512, 16, 8, 2048, 64
DK = D // H                      # 32
NCORES = 8
BC = B // NCORES                 # 4 sequences per core
T = BC * S                       # 1000 tokens per core
TP = 1008                        # padded free size
TCH = TP // 2                    # 504 t-chunk for GEMMs / LN
FCH = TP // 4                    # 252 t-chunk for FFN
NM = D // 128                    # 4 partition tiles per [D, T] tensor
NV = T // 125                    # 8 token-major v tiles of 125 tokens
EPS = 1e-5

N_LAYERS = int(os.environ.get("KERNEL_LAYERS", L))

# pvec column map (per-partition scalars, packed [4, 128, 16] per layer)
PC_BQ, PC_BK, PC_BO, PC_FB2 = 0, 1, 2, 3
PC_LAS, PC_LANS, PC_LAB = 4, 5, 6
PC_N1S, PC_N1NS, PC_N1B = 7, 8, 9
PC_N2S, PC_N2NS, PC_N2B = 10, 11, 12
PC_GB1, PC_GB2 = 13, 14
NPC = 16


def _build_program(n_layers: int):
    nc = bacc.Bacc("TRN2", target_bir_lowering=False, debug=False)

    # ---------------- DRAM I/O ----------------
    xT_d = nc.dram_tensor("xT", [IN, TP], F32, kind="ExternalInput")
    win_d = nc.dram_tensor("win", [IN, D], F32, kind="ExternalInput")
    add_d = nc.dram_tensor("addpos", [NM, 128, S], F32, kind="ExternalInput")
    ps_d = nc.dram_tensor("psrep", [128, S], F32, kind="ExternalInput")
    etb_d = nc.dram_tensor("etb", [2, 128, 256], BF16, kind="ExternalInput")
    e128_d = nc.dram_tensor("e128", [128, 128], F16, kind="ExternalInput")
    fpvec_d = nc.dram_tensor("fpvec", [NM, 128, 4], F32, kind="ExternalInput")
    out_d = nc.dram_tensor("out", [NM, 128, T], F32, kind="ExternalOutput")

    wl = []
    for l in range(n_layers):
        wl.append(dict(
            wq=nc.dram_tensor(f"L{l}_wq", [D, D], BF16, kind="ExternalInput"),
            wk=nc.dram_tensor(f"L{l}_wk", [D, D], BF16, kind="ExternalInput"),
            wv=nc.dram_tensor(f"L{l}_wv", [D, D], BF16, kind="ExternalInput"),
            wo=nc.dram_tensor(f"L{l}_wo", [D, D], BF16, kind="ExternalInput"),
            g1=nc.dram_tensor(f"L{l}_g1", [D, D // 2], BF16, kind="ExternalInput"),
            g2=nc.dram_tensor(f"L{l}_g2", [D // 2, 1], BF16, kind="ExternalInput"),
            f1=nc.dram_tensor(f"L{l}_f1", [D, DFF], BF16, kind="ExternalInput"),
            f2=nc.dram_tensor(f"L{l}_f2", [DFF, D], BF16, kind="ExternalInput"),
            pvec=nc.dram_tensor(f"L{l}_pvec", [NM, 128, NPC], F32, kind="ExternalInput"),
            fb1=nc.dram_tensor(f"L{l}_fb1", [128, 16], F32, kind="ExternalInput"),
            bvrep=nc.dram_tensor(f"L{l}_bvrep", [128, D], BF16, kind="ExternalInput"),
        ))

    with tile.TileContext(nc) as tc:
        import contextlib
        ctx = contextlib.ExitStack()
        with ctx:
            # ---------------- pools ----------------
            per = ctx.enter_context(tc.tile_pool(name="per", bufs=1))          # persistent
            wq_p = ctx.enter_context(tc.tile_pool(name="wq_p", bufs=1))        # weights (tags manage slots)
            sq_p = ctx.enter_context(tc.tile_pool(name="sq_p", bufs=1))
            ps_big = ctx.enter_context(tc.tile_pool(name="ps_big", bufs=1, space="PSUM"))
            ps_scav = ctx.enter_context(tc.tile_pool(name="ps_scav", bufs=1, space="PSUM"))
            ps_small = ctx.enter_context(tc.tile_pool(name="ps_small", bufs=1, space="PSUM"))

            def ptile(shape, dt, name, pool=per, tag=None, bufs=1):
                return pool.tile(shape, dt, name=name, tag=tag or name, bufs=bufs)

            # ---------------- persistent big buffers ----------------
            h_a = [ptile([128, TP], F32, f"h_a{m}") for m in range(NM)]
            h_b = [ptile([128, TP], F32, f"h_b{m}") for m in range(NM)]
            P = [ptile([128, TP], F32, f"P{m}") for m in range(NM)]       # blend / final-LN out
            h1 = [ptile([128, TP], F32, f"h1{m}") for m in range(NM)]
            h_bf = [ptile([128, TP], BF16, f"hbf{m}") for m in range(NM)]
            h1b = h_bf  # reused: h_bf is dead once the gate GEMM has consumed it
            q_t = [ptile([128, TP], BF16, f"q{m}") for m in range(NM)]
            k_t = [ptile([128, TP], BF16, f"k{m}") for m in range(NM)]
            ctx_t = [ptile([128, TP], BF16, f"ctx{m}") for m in range(NM)]
            v_t = [ptile([128, H * (DK + 1)], BF16, f"v{m}") for m in range(NV)]  # [125 tokens, 16*(32+1)]
            den_t = [ptile([128, TP], F16, f"den{m}") for m in range(NM)]

            # ---------------- constants ----------------
            ones128 = ptile([128, 1], F32, "ones128")
            nc.vector.memset(ones128[:], 1.0)
            ones1 = ptile([1, 128], F32, "ones1")
            nc.vector.memset(ones1[:], 1.0)
            ones128b = ptile([128, 1], BF16, "ones128b")
            nc.vector.memset(ones128b[:], 1.0)
            ones1h = ptile([1, 128], F16, "ones1h")
            nc.vector.memset(ones1h[:], 1.0)
            eps_t = ptile([1, 1], F32, "eps_t")
            nc.vector.memset(eps_t[:], EPS)
            e128_t = ptile([128, 128], F16, "e128_t")
            nc.sync.dma_start(e128_t[:], e128_d[:])
            etb_t = [ptile([128, 256], BF16, f"etb{c}") for c in range(2)]
            for c in range(2):
                nc.sync.dma_start(etb_t[c][:], etb_d[c])
            fpv_t = [ptile([128, 4], F32, f"fpv{m}") for m in range(NM)]
            for m in range(NM):
                nc.sync.dma_start(fpv_t[m][:], fpvec_d[m])

            # init: v ones-columns, denominators, pad columns of h_a
            for i in range(NV):
                nc.vector.memset(v_t[i][:125].rearrange("p (h c) -> p h c", h=H)[:, :, DK:DK + 1], 1.0)
            for m in range(NM):
                nc.vector.memset(den_t[m][:], 1.0)
                nc.vector.memset(ctx_t[m][:, T:TP], 0.0)
                nc.vector.memset(h_a[m][:, T:TP], 0.0)
                nc.vector.memset(h_b[m][:, T:TP], 0.0)
                nc.vector.memset(h1[m][:, T:TP], 0.0)
                nc.vector.memset(P[m][:, T:TP], 0.0)

            def r32(ap):
                return ap.bitcast(F32R)

            # ---------------- input projection + positional ----------------
            xT_t = [sq_p.tile([64, TCH], F32, name=f"xT{i}", tag="sq", bufs=3) for i in range(2)]
            for i in range(2):
                nc.sync.dma_start(xT_t[i][:], xT_d[:, i * TCH:(i + 1) * TCH])
            win_t = [wq_p.tile([IN, 128], F32, name=f"win{m}", tag="wqkvo", bufs=8) for m in range(NM)]
            for m in range(NM):
                nc.sync.dma_start(win_t[m][:], win_d[:, m * 128:(m + 1) * 128])
            for m in range(NM):
                for tci in range(2):
                    acc = ps_big.tile([128, TCH], F32, name="accin", tag="big", bufs=2)
                    nc.tensor.matmul(acc[:], win_t[m][:], xT_t[tci][:], start=True, stop=True)
                    nc.scalar.copy(h_a[m][:, tci * TCH:(tci + 1) * TCH], acc[:])
            ps_t = sq_p.tile([128, S], F32, name="ps_t", tag="sq", bufs=3)
            nc.sync.dma_start(ps_t[:], ps_d[:])
            for m in range(NM):
                add_t = sq_p.tile([128, S], F32, name=f"add{m}", tag="sq", bufs=3)
                nc.sync.dma_start(add_t[:], add_d[m])
                for b in range(BC):
                    sl = slice(b * S, (b + 1) * S)
                    nc.vector.tensor_mul(h_a[m][:, sl], h_a[m][:, sl], ps_t[:])
                    nc.vector.tensor_add(h_a[m][:, sl], h_a[m][:, sl], add_t[:])

            # ---------------- helpers ----------------
            def load_w(dram, kparts, ncols, dt, tag, bufs, name):
                """load [K, N] dram weight into kparts tiles of [128, ncols]."""
                ts = []
                for kc in range(kparts):
                    wt = wq_p.tile([128, ncols], dt, name=f"{name}_{kc}", tag=tag, bufs=bufs)
                    nc.sync.dma_start(wt[:], dram[kc * 128:(kc + 1) * 128, :])
                    ts.append(wt)
                return ts

            def gemm_fm_bf16(w_tiles, rhs_tiles, nm_out, out_cb, kparts=NM, tch=TCH,
                             tci_major=False):
                """feature-major GEMM: out[m*128:(m+1)*128, tchunk] tiles via PSUM.
                w_tiles: kparts tiles [128, nm_out*128] bf16 (lhsT slices taken per m)
                rhs_tiles: kparts activation tiles [128, TP] bf16
                out_cb(m, tci, acc_psum): consume psum [128, tch]"""
                nchunks = TP // tch
                order = [(m, tci) for tci in range(nchunks) for m in range(nm_out)] \
                    if tci_major else [(m, tci) for m in range(nm_out) for tci in range(nchunks)]
                for m, tci in order:
                    if True:
                        sl = slice(tci * tch, (tci + 1) * tch)
                        acc = ps_big.tile([128, tch], F32, name="acc", tag="big", bufs=2)
                        for kc in range(kparts):
                            nc.tensor.matmul(
                                acc[:], w_tiles[kc][:, m * 128:(m + 1) * 128],
                                rhs_tiles[kc][:, sl],
                                start=(kc == 0), stop=(kc == kparts - 1))
                        out_cb(m, tci, sl, acc)

            def emit_ln(x_tiles, out_tiles, pv_idx, pv_tiles=None, bf_mirror=None):
                """feature-major LayerNorm over D: out = (x-m)/sd*gamma+beta.
                pv_idx = (s_col, negs_col, b_col); pv_tiles: list of [128, NPC] tiles."""
                s_col, ns_col, b_col = pv_idx
                for tci in range(2):
                    sl = slice(tci * TCH, (tci + 1) * TCH)
                    ssum = ps_small.tile([1, TCH], F32, name="ssum", tag="small", bufs=3)
                    ssq = ps_small.tile([1, TCH], F32, name="ssq", tag="small", bufs=3)
                    for m in range(NM):
                        xb = sq_p.tile([128, TCH], BF16, name="xb", tag="sqb", bufs=6)
                        nc.scalar.copy(xb[:], x_tiles[m][:, sl])
                        sqb = sq_p.tile([128, TCH], BF16, name="sqb", tag="sqb", bufs=6)
                        nc.scalar.activation(sqb[:], x_tiles[m][:, sl], AF.Square)
                        nc.tensor.matmul(ssum[:], ones128b[:], xb[:],
                                         start=(m == 0), stop=(m == NM - 1))
                        nc.tensor.matmul(ssq[:], ones128b[:], sqb[:],
                                         start=(m == 0), stop=(m == NM - 1))
                    a0 = sq_p.tile([1, TCH], F32, name="a0", tag="lns", bufs=5)
                    a1 = sq_p.tile([1, TCH], F32, name="a1", tag="lns", bufs=5)
                    v1 = sq_p.tile([1, TCH], F32, name="v1", tag="lns", bufs=5)
                    rh = sq_p.tile([1, TCH], F16, name="rh", tag="lns", bufs=5)
                    mrh = sq_p.tile([1, TCH], F16, name="mrh", tag="lns", bufs=5)
                    nc.scalar.activation(a0[:], ssum[:], AF.Copy, scale=1.0 / D)
                    nc.scalar.activation(a1[:], ssq[:], AF.Copy, scale=1.0 / D)
                    nc.vector.tensor_mul(v1[:], a0[:], a0[:])
                    nc.vector.tensor_sub(v1[:], a1[:], v1[:])
                    nc.scalar.activation(v1[:], v1[:], AF.Sqrt, bias=eps_t[:])
                    with nc.allow_low_precision(reason="LN broadcast operands fp16"):
                        nc.vector.reciprocal(rh[:], v1[:])           # r = 1/sd (fp16)
                        nc.vector.tensor_mul(mrh[:], a0[:], rh[:])   # mr = mean*r (fp16)
                    rb = ps_small.tile([128, TCH], F32, name="rb", tag="small", bufs=3)
                    mrb = ps_small.tile([128, TCH], F32, name="mrb", tag="small", bufs=3)
                    nc.tensor.matmul(rb[:], ones1h[:], rh[:], start=True, stop=True)
                    nc.tensor.matmul(mrb[:], ones1h[:], mrh[:], start=True, stop=True)
                    rbs = sq_p.tile([128, TCH], F32, name="rbs", tag="d_t", bufs=3)
                    nc.scalar.copy(rbs[:], rb[:])
                    for m in range(NM):
                        pv = pv_tiles[m]
                        d_t = sq_p.tile([128, TCH], F32, name="d_t", tag="d_t", bufs=3)
                        nc.scalar.activation(d_t[:], mrb[:], AF.Identity,
                                             bias=pv[:, b_col:b_col + 1],
                                             scale=pv[:, ns_col:ns_col + 1])
                        if m < 2:
                            nc.vector.scalar_tensor_tensor(
                                out_tiles[m][:, sl], rb[:], pv[:, s_col:s_col + 1],
                                x_tiles[m][:, sl], op0=ALU.mult, op1=ALU.mult)
                            nc.vector.tensor_add(out_tiles[m][:, sl], out_tiles[m][:, sl], d_t[:])
                        else:
                            nc.gpsimd.scalar_tensor_tensor(
                                out_tiles[m][:, sl], rbs[:], pv[:, s_col:s_col + 1],
                                x_tiles[m][:, sl], op0=ALU.mult, op1=ALU.mult)
                            nc.gpsimd.tensor_add(out_tiles[m][:, sl], out_tiles[m][:, sl], d_t[:])
                        if bf_mirror is not None:
                            nc.gpsimd.tensor_copy(bf_mirror[m][:, sl], out_tiles[m][:, sl])

            # ---------------- layers ----------------
            for l in range(n_layers):
                W = wl[l]
                pv_t = []
                for m in range(NM):
                    pvt = wq_p.tile([128, NPC], F32, name=f"pv{l}_{m}", tag="pvec", bufs=8)
                    nc.sync.dma_start(pvt[:], W["pvec"][m])
                    pv_t.append(pvt)
                fb1_t = wq_p.tile([128, 16], F32, name=f"fb1_{l}", tag="fb1", bufs=2)
                nc.sync.dma_start(fb1_t[:], W["fb1"][:])
                bvr_t = wq_p.tile([128, D], BF16, name=f"bvr_{l}", tag="bvr", bufs=2)
                nc.sync.dma_start(bvr_t[:], W["bvrep"][:])

                if l == 0:
                    for m in range(NM):
                        nc.gpsimd.tensor_copy(h_bf[m][:], h_a[m][:])

                # ---- Q, K GEMMs (feature-major, bf16) ----
                wq_t = load_w(W["wq"], NM, D, BF16, "wqkvo", 8, f"wq{l}")

                def q_out(m, tci, sl, acc, _pv=pv_t):
                    nc.scalar.activation(q_t[m][:, sl], acc[:], AF.Identity,
                                         bias=_pv[m][:, PC_BQ:PC_BQ + 1])
                gemm_fm_bf16(wq_t, h_bf, NM, q_out)

                wk_t = load_w(W["wk"], NM, D, BF16, "wqkvo", 8, f"wk{l}")

                def k_out(m, tci, sl, acc, _pv=pv_t):
                    nc.scalar.activation(k_t[m][:, sl], acc[:], AF.Identity,
                                         bias=_pv[m][:, PC_BK:PC_BK + 1])
                gemm_fm_bf16(wk_t, h_bf, NM, k_out)

                # ---- V GEMM (token-major: lhsT = h_bf chunk, rhs = wv) ----
                wv_t = load_w(W["wv"], NM, D, BF16, "wqkvo", 8, f"wv{l}")
                for it in range(NV):
                    tsl = slice(it * 125, (it + 1) * 125)
                    acc = ps_big.tile([125, D], F32, name="accv", tag="big", bufs=2)
                    for kc in range(NM):
                        nc.tensor.matmul(acc[:], h_bf[kc][:, tsl], wv_t[kc][:],
                                         start=(kc == 0), stop=(kc == NM - 1))
                    nc.vector.tensor_add(
                        v_t[it][:125].rearrange("p (h c) -> p h c", h=H)[:, :, 0:DK],
                        acc[:].rearrange("p (h c) -> p h c", h=H),
                        bvr_t[:125].rearrange("p (h c) -> p h c", h=H))

                # ---- attention per (m-group, batch, head) ----
                # every head's AV matmul carries the v ones-column, so av row 32
                # is the softmax denominator; one fused [1,250] reciprocal puts
                # 1/den straight into the den tile at the head's row.
                for mt in range(NM):
                    stage_q = stage_k = None
                    for b in range(BC):
                        bsl = slice(b * S, (b + 1) * S)
                        for j in range(4):
                            h = mt * 4 + j
                            off = j * 32
                            if off == 96:
                                if stage_q is None:
                                    stage_q = sq_p.tile([32, TP], BF16, name="qs", tag="stage", bufs=2)
                                    stage_k = sq_p.tile([32, TP], BF16, name="ks", tag="stage", bufs=2)
                                    nc.gpsimd.tensor_copy(stage_q[:], q_t[mt][96:128, :])
                                    nc.gpsimd.tensor_copy(stage_k[:], k_t[mt][96:128, :])
                                q_src, k_src, soff = stage_q, stage_k, 0
                            else:
                                q_src, k_src, soff = q_t[mt], k_t[mt], off
                            av = ps_small.tile([33, S], F32, name="av", tag="small", bufs=3)
                            for c in range(2):
                                ksl = slice(b * S + c * 125, b * S + (c + 1) * 125)
                                sc = ps_scav.tile([125, S], F32, name="sc", tag="sc", bufs=3)
                                nc.tensor.matmul(sc[:], k_src[soff:soff + DK, ksl],
                                                 q_src[soff:soff + DK, bsl], start=True, stop=True)
                                es = sq_p.tile([125, S], BF16, name="es", tag="es", bufs=8)
                                nc.scalar.activation(es[:], sc[:], AF.Exp)
                                etb_eng = nc.vector if c == 0 else nc.gpsimd
                                etb_eng.tensor_mul(es[:], es[:], etb_t[c][:125, :S])
                                nc.tensor.matmul(av[:], v_t[b * 2 + c][:125, h * 33:h * 33 + 33],
                                                 es[:], start=(c == 0), stop=(c == 1))
                            nc.vector.tensor_copy(ctx_t[mt][off:off + 32, bsl], av[0:32, :])
                            with nc.allow_low_precision(reason="softmax denominators fp16"):
                                nc.vector.reciprocal(den_t[mt][off:off + 1, bsl], av[32:33, :])
                    # broadcast reciprocals to all head rows, scale ctx
                    for tci in range(2):
                        sl = slice(tci * TCH, (tci + 1) * TCH)
                        rbm = ps_small.tile([128, TCH], F32, name="rbm", tag="small", bufs=3)
                        nc.tensor.matmul(rbm[:], e128_t[:], den_t[mt][:, sl],
                                         start=True, stop=True)
                        nc.vector.tensor_mul(ctx_t[mt][:, sl], ctx_t[mt][:, sl], rbm[:])

                # ---- gate: g = sigmoid(relu(h@g1+gb1) @ g2 + gb2) ----
                g1_t = load_w(W["g1"], NM, D // 2, BF16, "wg1", 6, f"g1{l}")
                relu_t = {}

                def g1_out(m, tci, sl, acc, _pv=pv_t, _rt=relu_t):
                    rt = sq_p.tile([128, TCH], BF16, name="relu", tag="relu", bufs=4)
                    nc.scalar.activation(rt[:], acc[:], AF.Relu,
                                         bias=_pv[m][:, PC_GB1:PC_GB1 + 1])
                    _rt[(m, tci)] = rt
                gemm_fm_bf16(g1_t, h_bf, 2, g1_out)
                g2_t = load_w(W["g2"], 2, 1, BF16, "pvec", 8, f"g2{l}")
                g_chunks = []
                for tci in range(2):
                    sl = slice(tci * TCH, (tci + 1) * TCH)
                    gacc = ps_small.tile([1, TCH], F32, name="gacc", tag="small", bufs=3)
                    for kc in range(2):
                        nc.tensor.matmul(gacc[:], g2_t[kc][:], relu_t[(kc, tci)][:],
                                         start=(kc == 0), stop=(kc == 1))
                    gch = sq_p.tile([1, TCH], F16, name="gch", tag="lns", bufs=5)
                    nc.scalar.activation(gch[:], gacc[:], AF.Sigmoid,
                                         bias=pv_t[0][0:1, PC_GB2:PC_GB2 + 1])
                    g_chunks.append(gch)

                # ---- O GEMM + gated blend: P = g*(o+bo-h) + h ----
                wo_t = load_w(W["wo"], NM, D, BF16, "wqkvo", 8, f"wo{l}")
                gb_ps = []
                for tci in range(2):
                    sl = slice(tci * TCH, (tci + 1) * TCH)
                    gb = ps_small.tile([128, TCH], F32, name="gb", tag="small", bufs=3)
                    nc.tensor.matmul(gb[:], ones1h[:], g_chunks[tci][:], start=True, stop=True)
                    gbs = sq_p.tile([128, TCH], F32, name="gbs", tag="d_t", bufs=3)
                    nc.scalar.copy(gbs[:], gb[:])
                    gb_ps.append(gbs)

                def o_out(m, tci, sl, acc, _pv=pv_t, _gb=gb_ps):
                    op = sq_p.tile([128, TCH], F32, name="op", tag="d_t", bufs=3)
                    nc.scalar.activation(op[:], acc[:], AF.Identity,
                                         bias=_pv[m][:, PC_BO:PC_BO + 1])   # o + bo
                    nc.gpsimd.tensor_sub(P[m][:, sl], op[:], h_a[m][:, sl])
                    nc.gpsimd.tensor_mul(P[m][:, sl], P[m][:, sl], _gb[tci][:])
                    nc.gpsimd.tensor_add(P[m][:, sl], P[m][:, sl], h_a[m][:, sl])
                gemm_fm_bf16(wo_t, ctx_t, NM, o_out, tci_major=True)

                # ---- attn_out = LN_a(P) (in-place); h_a += attn_out; h1 = LN_1(h_a) ----
                emit_ln(P, P, (PC_LAS, PC_LANS, PC_LAB), pv_t)
                for m in range(NM):
                    nc.vector.tensor_add(h_a[m][:], h_a[m][:], P[m][:])
                emit_ln(h_a, h1, (PC_N1S, PC_N1NS, PC_N1B), pv_t, bf_mirror=h1b)

                # ---- FFN: h_a = h1 + gelu(h1@f1+fb1)@f2 + fb2 ; h_next = LN_2 ----
                f1_t = []
                for kc in range(NM):
                    for hh in range(2):
                        wt = wq_p.tile([128, DFF // 2], BF16, name=f"f1{l}_{kc}_{hh}",
                                       tag="wf1", bufs=9)
                        nc.sync.dma_start(wt[:], W["f1"][kc * 128:(kc + 1) * 128,
                                                         hh * (DFF // 2):(hh + 1) * (DFF // 2)])
                        f1_t.append(wt)
                f2_t = load_w(W["f2"], DFF // 128, D, BF16, "wf2", 17, f"f2{l}")
                for tci in range(4):
                    sl = slice(tci * FCH, (tci + 1) * FCH)
                    gelu_t = []
                    for mf in range(DFF // 128):
                        acc = ps_big.tile([128, FCH], F32, name="accf1", tag="big", bufs=2)
                        for kc in range(NM):
                            wt = f1_t[kc * 2 + (mf // 8)]
                            csl = slice((mf % 8) * 128, (mf % 8 + 1) * 128)
                            nc.tensor.matmul(acc[:], wt[:, csl],
                                             h1b[kc][:, sl], start=(kc == 0), stop=(kc == NM - 1))
                        gt = sq_p.tile([128, FCH], BF16, name="gelu", tag="gelu", bufs=17)
                        nc.scalar.activation(gt[:], acc[:], AF.Gelu,
                                             bias=fb1_t[:, mf:mf + 1])
                        gelu_t.append(gt)
                    for m in range(NM):
                        acc = ps_big.tile([128, FCH], F32, name="accf2", tag="big", bufs=2)
                        for kc in range(DFF // 128):
                            nc.tensor.matmul(acc[:], f2_t[kc][:, m * 128:(m + 1) * 128],
                                             gelu_t[kc][:], start=(kc == 0), stop=(kc == DFF // 128 - 1))
                        # h_a[m] = (ff2 + fb2) + h1
                        nc.vector.scalar_tensor_tensor(
                            h_a[m][:, sl], acc[:], pv_t[m][:, PC_FB2:PC_FB2 + 1],
                            h1[m][:, sl], op0=ALU.add, op1=ALU.add)
                emit_ln(h_a, h_b, (PC_N2S, PC_N2NS, PC_N2B), pv_t, bf_mirror=h_bf)
                h_a, h_b = h_b, h_a

            # ---------------- final LN + output ----------------
            emit_ln(h_a, P, (0, 1, 2), fpv_t)
            for m in range(NM):
                nc.sync.dma_start(out_d[m], P[m][:, :T])

    nc.compile()
    return nc


@functools.lru_cache(maxsize=2)
def _get_program(n_layers):
    return _build_program(n_layers)


def _prep_common(w_in, b_in, pe, pos_scale, pos_bias, tbias, layers,
                 final_scale, final_bias, n_layers):
    """host-side constant prep -> dict of common in_map entries."""
    cm = {}
    cm["win"] = np.ascontiguousarray(w_in, np.float32)
    # positional: h = (h0 + b_in + pe) * ps + pb ; ADD = (b_in + pe)*ps + pb
    ps = np.asarray(pos_scale, np.float32).reshape(S, 1)
    pb = np.asarray(pos_bias, np.float32).reshape(S, 1)
    add = ((np.asarray(b_in, np.float32)[None, :] + np.asarray(pe, np.float32)) * ps + pb).T  # [D, S]
    cm["addpos"] = np.ascontiguousarray(add.reshape(NM, 128, S), np.float32)
    cm["psrep"] = np.ascontiguousarray(np.broadcast_to(ps.T, (128, S)), np.float32)
    etbT = np.exp(np.asarray(tbias, np.float32)).T  # [sk, sq]
    etb = np.zeros((2, 128, 256), BF16NP)
    for c in range(2):
        etb[c, :125, :S] = etbT[c * 125:(c + 1) * 125, :].astype(BF16NP)
    cm["etb"] = etb
    e128 = np.zeros((128, 128), np.float16)
    for p in range(128):
        e128[(p // 32) * 32, p] = np.float16(1.0)
    cm["e128"] = e128
    fpv = np.zeros((NM, 128, 4), np.float32)
    fs = np.asarray(final_scale, np.float32).reshape(NM, 128)
    fb = np.asarray(final_bias, np.float32).reshape(NM, 128)
    fpv[:, :, 0] = fs
    fpv[:, :, 1] = -fs
    fpv[:, :, 2] = fb
    cm["fpvec"] = fpv

    scale = 1.0 / math.sqrt(DK)
    ly = {k: np.asarray(v) for k, v in layers.items()}
    for l in range(n_layers):
        cm[f"L{l}_wq"] = (ly["wq"][l] * scale).astype(BF16NP)
        cm[f"L{l}_wk"] = ly["wk"][l].astype(BF16NP)
        cm[f"L{l}_wv"] = ly["wv"][l].astype(BF16NP)
        cm[f"L{l}_wo"] = ly["wo"][l].astype(BF16NP)
        cm[f"L{l}_g1"] = ly["g1"][l].astype(BF16NP)
        cm[f"L{l}_g2"] = ly["g2"][l].astype(BF16NP)
        cm[f"L{l}_f1"] = ly["f1"][l].astype(BF16NP)
        cm[f"L{l}_f2"] = ly["f2"][l].astype(BF16NP)
        pvec = np.zeros((NM, 128, NPC), np.float32)
        pvec[:, :, PC_BQ] = (ly["bq"][l] * scale).reshape(NM, 128)
        pvec[:, :, PC_BK] = ly["bk"][l].reshape(NM, 128)
        pvec[:, :, PC_BO] = ly["bo"][l].reshape(NM, 128)
        pvec[:, :, PC_FB2] = ly["fb2"][l].reshape(NM, 128)
        for (cs, cns, cb), nm_ in [((PC_LAS, PC_LANS, PC_LAB), ("lna_s", "lna_b")),
                                   ((PC_N1S, PC_N1NS, PC_N1B), ("n1s", "n1b")),
                                   ((PC_N2S, PC_N2NS, PC_N2B), ("n2s", "n2b"))]:
            sv = ly[nm_[0]][l].reshape(NM, 128)
            bv_ = ly[nm_[1]][l].reshape(NM, 128)
            pvec[:, :, cs] = sv
            pvec[:, :, cns] = -sv
            pvec[:, :, cb] = bv_
        gb1 = ly["gb1"][l].reshape(2, 128)
        pvec[0:2, :, PC_GB1] = gb1
        pvec[0, 0, PC_GB2] = float(ly["gb2"][l].reshape(-1)[0])
        cm[f"L{l}_pvec"] = pvec
        cm[f"L{l}_fb1"] = np.ascontiguousarray(ly["fb1"][l].reshape(16, 128).T, np.float32)
        cm[f"L{l}_bvrep"] = np.broadcast_to(
            ly["bv"][l].astype(BF16NP), (128, D)).copy()
    return cm


def make_in_maps(x, **consts):
    """build the 8 per-core input maps (full inputs -> shards)."""
    n_layers = N_LAYERS
    cm = _prep_common(n_layers=n_layers, **consts)
    x = np.asarray(x, np.float32)
    in_maps = []
    for i in range(NCORES):
        shard = x[i * BC:(i + 1) * BC]                    # [4, 250, 64]
        xT = np.zeros((IN, TP), np.float32)
        xT[:, :T] = shard.reshape(T, IN).T
        m = dict(cm)
        m["xT"] = xT
        in_maps.append(m)
    return in_maps


def kernel(x, w_in, b_in, pe, pos_scale, pos_bias, tbias, layers,
           final_scale, final_bias):
    nc = _get_program(N_LAYERS)
    in_maps = make_in_maps(
        x, w_in=w_in, b_in=b_in, pe=pe, pos_scale=pos_scale,
        pos_bias=pos_bias, tbias=tbias, layers=layers,
        final_scale=final_scale, final_bias=final_bias)
    res = run_bass_kernel_spmd(nc, in_maps, core_ids=list(range(NCORES)))
    outs = []
    for i in range(NCORES):
        o = np.asarray(res.results[i]["out"])             # [4, 128, 1000]
        o = o.reshape(D, BC, S).transpose(1, 2, 0)        # [4, 250, 512]
        outs.append(o)
    return np.concatenate(outs, axis=0).astype(np.float32)
